# revision 1
# baseline (speedup 1.0000x reference)
"""Bass/Trainium2 kernel for nn_BaseAttention (B=2, S=2048, H=1024, NH=16, HD=64).

Sharding: 8 cores = 2 batches x 4 head-groups (4 heads each core).
Each core computes, for its (batch b, head-group hb):
    qkv slice -> attention over packed masked keys -> partial out-projection
and writes partial^T [H, S].  Host sums the 4 partials per batch and
transposes.

v3 design (all-bf16 data path; cost-model-guided):
  * All matmul inputs bf16: 1 cyc/row at any moving size, and input DMA is
    half of f32.  PSUM accumulation stays f32.  Weights-load cost (separate
    Ldweights on the PE sequencer, ~47ns) is amortized by keeping every
    matmul's moving dim at 512.
  * Masked keys packed on host (KP = ceil(max_count/128)*128); 1/sqrt(HD)
    folded into wq on the host; key-padding bias fused into exp.
  * Scores in S^T layout [key_part, q_free].  exp on the Act engine for most
    key-tiles; for kt in DVE_KT the exp is one DVE tensor_scalar op
    (Schraudolph: uint16(round(s*A + B)) bits ARE bf16(exp(s)); saturates to
    +0 for masked keys).  This splits the softmax-exp wall (~72us Act-only)
    across two engines.
  * AV as P^T @ V_aug with a 65th ones column giving the softmax denominator
    for free; normalization reads the AV PSUM directly (reciprocal + gpsimd
    partition_broadcast + multiply) and writes normalized A^T to SBUF - no
    separate PSUM evacuation pass.
  * Issue order is software-pipelined: scores(kt+1) is issued before AV(kt)
    so the in-order PE stream never stalls on the exp of the current tile.
"""

import numpy as np
import ml_dtypes

import concourse.bass as bass
import concourse.mybir as mybir
import concourse.tile as tile
from concourse import bacc
from concourse import bass_utils

B, S, H = 2, 2048, 1024
NH, HD = 16, 64
SCALE = HD ** -0.5
NCORES = 8
CPB = NCORES // B          # cores per batch = 4
NHL = NH // CPB            # local heads per core = 4
QD = NHL * HD              # local head-dim total = 256
HT = H // 128              # k-tiles over hidden dim = 8
MT = QD // 128             # partition-tiles over local head dims = 2

F32 = mybir.dt.float32
BF16 = mybir.dt.bfloat16
U16 = mybir.dt.uint16
BF = ml_dtypes.bfloat16

# Schraudolph exp -> bf16 bits: u16 = round(s * AEXP + (bias*AEXP + BEXP)),
# saturating at 0.  AEXP = 2^7/ln2 (bf16 exponent LSB is bit 7); -6 tunes
# the fraction bias (max rel err ~3.5%, mean ~1.7%).
AEXP = 128.0 / float(np.log(2.0))
BEXP = 127.0 * 128.0 - 6.0
DVE_KT = (1, 4, 7)         # key-tiles whose exp runs on DVE via Schraudolph


def _chunks(total, size):
    out = []
    o = 0
    while o < total:
        c = min(size, total - o)
        out.append((o, c))
        o += c
    return out


def build_kernel(KP):
    KT = KP // 128
    nc = bacc.Bacc("TRN2")
    xT = nc.dram_tensor("xT", [H, S], BF16, kind="ExternalInput")
    xpT = nc.dram_tensor("xpT", [H, KP], BF16, kind="ExternalInput")
    wqT = nc.dram_tensor("wqT", [H, QD], BF16, kind="ExternalInput")
    wkT = nc.dram_tensor("wkT", [H, QD], BF16, kind="ExternalInput")
    wvT = nc.dram_tensor("wvT", [H, QD], BF16, kind="ExternalInput")
    woT = nc.dram_tensor("woT", [QD, H], BF16, kind="ExternalInput")
    bk = nc.dram_tensor("bk", [128, KT], F32, kind="ExternalInput")
    bk2 = nc.dram_tensor("bk2", [128, KT], F32, kind="ExternalInput")
    outT = nc.dram_tensor("outT", [H, S], BF16, kind="ExternalOutput")

    with tile.TileContext(nc) as tc:
        with tile.TileContext.tile_pool(tc, name="wts", bufs=1) as wp:
            wq_sb = wp.tile([128, HT, QD], BF16)
            wk_sb = wp.tile([128, HT, QD], BF16)
            wv_sb = wp.tile([128, HT, QD], BF16)
            wo_sb = wp.tile([128, MT, H], BF16)
            bk_sb = wp.tile([128, KT], F32)
            bk2_sb = wp.tile([128, KT], F32)
            xT_sb = wp.tile([128, HT, S], BF16)
            xpT_sb = wp.tile([128, HT, KP], BF16)
            qT_sb = wp.tile([128, MT, S], BF16)
            kT_sb = wp.tile([128, MT, KP], BF16)
            va_sb = wp.tile([128, KT, NHL, 65], BF16)   # V rows + ones col
            aT_sb = wp.tile([128, MT, S], BF16)

            # --- input DMA: K-path first (wk/xpT gate the K projection),
            # then wv (V proj), wq, xT (Q proj), wo last.
            # weights as single multi-dim DMAs (HWDGE issue costs ~630ns
            # per DMA instruction); xpT/xT stay per-tile so the K/Q
            # projections chase their arrival.
            nc.sync.dma_start(out=wk_sb,
                              in_=wkT.ap().rearrange("(t p) d -> p t d",
                                                     p=128))
            nc.scalar.dma_start(out=bk_sb, in_=bk.ap())
            nc.scalar.dma_start(out=bk2_sb, in_=bk2.ap())
            nc.scalar.dma_start(out=wv_sb,
                                in_=wvT.ap().rearrange("(t p) d -> p t d",
                                                       p=128))
            nc.scalar.dma_start(out=wq_sb,
                                in_=wqT.ap().rearrange("(t p) d -> p t d",
                                                       p=128))
            for ht in range(HT):
                nc.sync.dma_start(out=xpT_sb[:, ht, :],
                                  in_=xpT.ap()[ht * 128:(ht + 1) * 128, :])
            for ht in range(HT):
                nc.sync.dma_start(out=xT_sb[:, ht, :],
                                  in_=xT.ap()[ht * 128:(ht + 1) * 128, :])
            nc.sync.dma_start(out=wo_sb,
                              in_=woT.ap().rearrange("(t p) d -> p t d",
                                                     p=128))
            nc.vector.memset(va_sb[:, :, :, 64:65], 1.0)

            # shared [128,1024] f32 psum pool: K-proj / Q-proj / scores / out-proj
            with tile.TileContext.tile_pool(tc, name="pss", bufs=2,
                                            space="PSUM") as pss:
                evac_flip = [0]

                def evac(dst, src):
                    # alternate psum evacuations between Act and DVE
                    if evac_flip[0] % 2 == 0:
                        nc.scalar.copy(dst, src)
                    else:
                        nc.vector.tensor_copy(dst, src)
                    evac_flip[0] += 1

                with tile.TileContext.tile_pool(tc, name="psk", bufs=2,
                                                space="PSUM") as psk, \
                     tile.TileContext.tile_pool(tc, name="pv", bufs=2,
                                                space="PSUM") as pvp:
                    # ---- K^T projection, ht-outer so matmuls chase the DMA.
                    kchunks = []
                    for mt in range(MT):
                        for po, pw in _chunks(KP, 1024):
                            pool, tag = (pss, "pss") if pw > 128 else (psk, "psk")
                            ps = pool.tile([128, min(pw, 1024)], F32, tag=tag,
                                           name=f"ps_k{mt}_{po}")
                            kchunks.append((mt, po, pw, ps))
                    for ht in range(HT):
                        for mt, po, pw, ps in kchunks:
                            for co, cw in _chunks(pw, 512):
                                nc.tensor.matmul(
                                    ps[:, co:co + cw],
                                    wk_sb[:, ht, mt * 128:(mt + 1) * 128],
                                    xpT_sb[:, ht, po + co:po + co + cw],
                                    start=(ht == 0), stop=(ht == HT - 1))
                    for mt, po, pw, ps in kchunks:
                        evac(kT_sb[:, mt, po:po + pw], ps[:, 0:pw])

                    # ---- V projection (keys on partitions)
                    for st in range(KT):
                        pv = pvp.tile([128, QD], F32, tag="pv", name="ps_v")
                        for ht in range(HT):
                            nc.tensor.matmul(
                                pv, xpT_sb[:, ht, st * 128:(st + 1) * 128],
                                wv_sb[:, ht, :],
                                start=(ht == 0), stop=(ht == HT - 1))
                        nc.vector.tensor_copy(
                            va_sb[:, st, :, 0:64],
                            pv.rearrange("p (h d) -> p h d", h=NHL))

                    # ---- Q^T projection, first chunk only (heads 0/1, po=0);
                    # the other three chunks are emitted as PE filler at the
                    # attention pair transitions, one pair before they're used.
                    def emit_q(mt, poi, on_pool=False):
                        po = poi * 1024
                        ps = pss.tile([128, 1024], F32, tag="pss",
                                      name=f"ps_q{mt}_{poi}")
                        for ht in range(HT):
                            for co, cw in _chunks(1024, 512):
                                nc.tensor.matmul(
                                    ps[:, co:co + cw],
                                    wq_sb[:, ht, mt * 128:(mt + 1) * 128],
                                    xT_sb[:, ht, po + co:po + co + cw],
                                    start=(ht == 0), stop=(ht == HT - 1))
                        if on_pool:
                            nc.gpsimd.tensor_copy(
                                qT_sb[:, mt, po:po + 1024], ps)
                        else:
                            evac(qT_sb[:, mt, po:po + 1024], ps)

                    emit_q(0, 0)
                    emit_q(1, 0)
                    emit_q(0, 1)
                    emit_q(1, 1)

                # ---- attention: two (head, po) blocks in lockstep per kt so
                # the 2-deep scores-psum rotation always has 2 scores + 2 AV
                # matmuls (1708ns PE) between an exp's issue and the reuse of
                # its psum slot (~1200ns exp service) - the in-order PE never
                # stalls on the Act/DVE exp.  AV is deferred 2 kt behind its
                # scores (pex SBUF tiles buffer the gap).  DVE tile sets are
                # staggered between the two blocks so Act never backlogs.
                with tile.TileContext.tile_pool(tc, name="po", bufs=2,
                                                space="PSUM") as pop, \
                     tile.TileContext.tile_pool(tc, name="pex", bufs=10) as pxp, \
                     tile.TileContext.tile_pool(tc, name="bc", bufs=3) as bcp, \
                     tile.TileContext.tile_pool(tc, name="rc", bufs=3) as rcp:
                    pairs = [((0, 0), (1, 0)), ((2, 0), (3, 0)),
                             ((0, 1), (1, 1)), ((2, 1), (3, 1))]
                    # Q chunk needed by pair i+1, emitted as PE filler in the
                    # pair-i -> pair-i+1 transition (PE idles there waiting on
                    # the normalize chain to free the AV psums).
                    fillers = [None, None, None, None]
                    for pair, filler in zip(pairs, fillers):
                        blocks = []
                        for i, (h, poi) in enumerate(pair):
                            blocks.append({
                                "h": h, "mtq": (h * HD) // 128,
                                "rb": (h * HD) % 128, "po": poi * 1024,
                                "pso": pop.tile([HD + 1, 1024], F32, tag="po",
                                                name=f"ps_o{h}_{poi}"),
                                "pend": [],
                                "dve": DVE_KT if i == 0 else tuple(
                                    k + 1 for k in DVE_KT),
                            })

                        def flush_av(blk):
                            pkt, ppx = blk["pend"].pop(0)
                            for co, cw in _chunks(1024, 512):
                                nc.tensor.matmul(
                                    blk["pso"][:, co:co + cw],
                                    va_sb[:, pkt, blk["h"], :],
                                    ppx[:, co:co + cw],
                                    start=(pkt == 0), stop=(pkt == KT - 1))

                        for kt in range(KT):
                            for blk in blocks:
                                h, po = blk["h"], blk["po"]
                                ps = pss.tile([128, 1024], F32, tag="pss",
                                              name=f"ps_s{h}_{po}_{kt}")
                                for co, cw in _chunks(1024, 512):
                                    q0 = po + co
                                    nc.tensor.matmul(
                                        ps[:, co:co + cw],
                                        kT_sb[blk["rb"]:blk["rb"] + HD,
                                              blk["mtq"],
                                              kt * 128:(kt + 1) * 128],
                                        qT_sb[blk["rb"]:blk["rb"] + HD,
                                              blk["mtq"], q0:q0 + cw],
                                        start=True, stop=True)
                                px = pxp.tile([128, 1024], BF16, tag="pex",
                                              name=f"pex{h}_{po}_{kt}")
                                if kt in blk["dve"]:
                                    nc.vector.tensor_scalar(
                                        px.bitcast(U16), ps,
                                        AEXP, bk2_sb[:, kt:kt + 1],
                                        mybir.AluOpType.mult,
                                        mybir.AluOpType.add)
                                else:
                                    nc.scalar.activation(
                                        out=px, in_=ps,
                                        func=mybir.ActivationFunctionType.Exp,
                                        bias=bk_sb[:, kt:kt + 1], scale=1.0)
                                blk["pend"].append((kt, px))
                            for blk in blocks:
                                if len(blk["pend"]) > 3:
                                    flush_av(blk)
                        for blk in blocks:
                            while blk["pend"]:
                                flush_av(blk)
                            # normalize straight out of PSUM, per 512-col
                            # half: aT = pso[0:64] / broadcast(pso[64])
                            h, poi = blk["h"], blk["po"]
                            pso = blk["pso"]
                            rc = rcp.tile([1, 1024], F32, tag="rc",
                                          name=f"rc{h}_{poi}")
                            nc.vector.reciprocal(rc, pso[HD:HD + 1, :])
                            bc = bcp.tile([HD, 1024], F32, tag="bc",
                                          name=f"bc{h}_{poi}")
                            nc.gpsimd.partition_broadcast(bc, rc)
                            nc.vector.tensor_mul(
                                aT_sb[blk["rb"]:blk["rb"] + HD, blk["mtq"],
                                      blk["po"]:blk["po"] + 1024],
                                pso[0:HD, :], bc)

                # ---- out-projection: partial^T[j, q] = W_o-slice^T . A^T
                # (attention pools are closed here: use a second psum pool so
                # the chunk rotation is 4 deep and never waits on staging)
                with tile.TileContext.tile_pool(tc, name="stg", bufs=6) as sgp, \
                     tile.TileContext.tile_pool(tc, name="pf2", bufs=2,
                                                space="PSUM") as pf2:
                    di = 0
                    for jt in range(HT):
                        for ho, hw in _chunks(S, 1024):
                            # first chunks draw from the always-open pss
                            # pool: the pf2 pool's opening waits on the
                            # attention pools' release (last pair's
                            # normalize chain)
                            if di < 2 or di % 2 == 0:
                                pf = pss.tile([128, 1024], F32, tag="pss",
                                              name=f"ps_f{jt}_{ho}")
                            else:
                                pf = pf2.tile([128, 1024], F32, tag="pf2",
                                              name=f"ps_f{jt}_{ho}")
                            for mt in range(MT):
                                for co, cw in _chunks(hw, 512):
                                    nc.tensor.matmul(
                                        pf[:, co:co + cw],
                                        wo_sb[:, mt, jt * 128:(jt + 1) * 128],
                                        aT_sb[:, mt, ho + co:ho + co + cw],
                                        start=(mt == 0), stop=(mt == MT - 1))
                            stg = sgp.tile([128, 1024], BF16, tag="stg",
                                           name="stage")
                            if di >= 16:
                                # tail chunks: halves on both engines with
                                # separate DMAs so the final write is short
                                nc.scalar.copy(stg[:, 0:512], pf[:, 0:512])
                                nc.vector.tensor_copy(stg[:, 512:1024],
                                                      pf[:, 512:1024])
                                for co in (0, 512):
                                    nc.sync.dma_start(
                                        out=outT.ap()[jt * 128:(jt + 1) * 128,
                                                      ho + co:ho + co + 512],
                                        in_=stg[:, co:co + 512])
                            else:
                                if di % 2 == 0:
                                    nc.scalar.copy(stg, pf[:, 0:hw])
                                else:
                                    nc.vector.tensor_copy(stg, pf[:, 0:hw])
                                nc.sync.dma_start(
                                    out=outT.ap()[jt * 128:(jt + 1) * 128,
                                                  ho:ho + hw],
                                    in_=stg)
                            di += 1

    nc.compile()
    return nc


def _prep_inputs(hidden_states, attention_mask, w_qkv, w_out):
    """Shard + transpose + quantize inputs for the 8 cores."""
    hs = np.asarray(hidden_states, dtype=np.float32)
    mask = np.asarray(attention_mask)
    wqkv = np.asarray(w_qkv, dtype=np.float32)
    wo = np.asarray(w_out, dtype=np.float32)

    idxs = [np.nonzero(mask[b] != 0)[0] for b in range(B)]
    counts = [len(ix) for ix in idxs]
    KP = max(128, ((max(counts) + 127) // 128) * 128)
    KT = KP // 128

    xTs, xpTs, bks, bk2s = [], [], [], []
    for b in range(B):
        xb = hs[b].astype(BF)
        xTs.append(np.ascontiguousarray(xb.T))
        xp = np.zeros((KP, H), dtype=BF)
        xp[:counts[b]] = xb[idxs[b]]
        xpTs.append(np.ascontiguousarray(xp.T))
        bias = np.zeros(KP, dtype=np.float32)
        bias[counts[b]:] = -30000.0
        bias = np.ascontiguousarray(bias.reshape(KT, 128).T)
        bks.append(bias)
        bk2s.append(np.ascontiguousarray(
            (bias * AEXP + BEXP).astype(np.float32)))

    in_maps = []
    for c in range(NCORES):
        b, hb = c // CPB, c % CPB
        sl = slice(hb * QD, (hb + 1) * QD)
        in_maps.append({
            "xT": xTs[b],
            "xpT": xpTs[b],
            "wqT": np.ascontiguousarray(
                (wqkv[sl, :] * SCALE).astype(BF).T),
            "wkT": np.ascontiguousarray(
                wqkv[H + sl.start:H + sl.stop, :].astype(BF).T),
            "wvT": np.ascontiguousarray(
                wqkv[2 * H + sl.start:2 * H + sl.stop, :].astype(BF).T),
            "woT": np.ascontiguousarray(wo[:, sl].astype(BF).T),
            "bk": bks[b],
            "bk2": bk2s[b],
        })
    return KP, in_maps


_NC_CACHE = {}


def kernel(hidden_states, attention_mask, w_qkv, w_out):
    KP, in_maps = _prep_inputs(hidden_states, attention_mask, w_qkv, w_out)
    if KP not in _NC_CACHE:
        _NC_CACHE[KP] = build_kernel(KP)
    nc = _NC_CACHE[KP]
    res = bass_utils.run_bass_kernel_spmd(nc, in_maps,
                                          core_ids=list(range(NCORES)))
    out = np.empty((B, S, H), dtype=np.float32)
    for b in range(B):
        acc = res.results[b * CPB]["outT"].astype(np.float32).copy()
        for c in range(b * CPB + 1, (b + 1) * CPB):
            acc += res.results[c]["outT"]
        out[b] = acc.T
    return out



# revision 32
# speedup vs baseline: 1.2500x; 1.2500x over previous
"""Bass/Trainium2 kernel for nn_BaseAttention (B=2, S=2048, H=1024, NH=16, HD=64).

Sharding: 8 cores = 2 batches x 4 head-groups (4 heads each core).
Each core computes, for its (batch b, head-group hb):
    qkv slice -> attention over packed masked keys -> partial out-projection
and writes partial^T [H, S].  Host sums the 4 partials per batch and
transposes.

v4 design (all-fp16 data path; cost-model-guided):
  * fp16 everywhere instead of bf16: same 1 cyc/row matmul throughput and
    identical DMA bytes, but 8x less quantization error -- the error budget
    is then dominated by the Schraudolph exp tiles alone.
  * AV computed in [q, d] orientation: stationary = exp'd score tile
    [128 keys, 128 q], moving = V-augmented [128 keys, 65] (65th column of
    ones gives the softmax denominator).  Cost-model matmul time is
    out_free x 1 cyc, so AV drops ~2x vs the [d^T, q] orientation.
    Normalization becomes a per-partition tensor op (reciprocal of the
    denominator column + broadcast multiply), and the [q, d] -> [d, q]
    transpose needed by the out-projection runs on the DMA XBAR
    (dma_start_transpose), costing no PE/ACT/DVE time and no PSUM banks.
  * Scores in S^T layout [key_part, q_free], exp split per 512-half across
    Act (accurate exp) / DVE / Pool (Schraudolph: uint16(round(s*A + B))
    bits ARE fp16(exp(s)); saturates to +0 for masked keys).  Splitting
    halves keeps the exp service latency under the scores-psum reuse window.
  * Out-projection for queries 0:1024 runs between the two attention halves
    so its output DMA overlaps the second half's compute; Q-projection for
    queries 1024:2048 is PE filler in the same slot.
  * Masked keys packed on host (KP = ceil(max_count/128)*128); 1/sqrt(HD)
    folded into wq on the host; key-padding bias fused into exp.
"""

import numpy as np

import concourse.bass as bass
import concourse.mybir as mybir
import concourse.tile as tile
from concourse import bacc
from concourse import bass_utils

B, S, H = 2, 2048, 1024
NH, HD = 16, 64
SCALE = HD ** -0.5
NCORES = 8
CPB = NCORES // B          # cores per batch = 4
NHL = NH // CPB            # local heads per core = 4
QD = NHL * HD              # local head-dim total = 256
HT = H // 128              # k-tiles over hidden dim = 8
MT = QD // 128             # partition-tiles over local head dims = 2
PO = 2                     # query halves (1024 each)
QT = 8                     # 128-query tiles per half

F32 = mybir.dt.float32
F16 = mybir.dt.float16
U16 = mybir.dt.uint16
NPF16 = np.float16

# Schraudolph exp -> fp16 bits: u16 = round(s * AEXP + BEXP), saturating at
# 0.  AEXP = 2^10/ln2 (fp16 exponent LSB is bit 10); the -60 fraction-bias
# offset minimizes the max relative error over the sawtooth.
AEXP = 1024.0 / float(np.log(2.0))
BEXP = 15.0 * 1024.0 - 60.0


def _chunks(total, size):
    out = []
    o = 0
    while o < total:
        c = min(size, total - o)
        out.append((o, c))
        o += c
    return out


# Engines for the two 512-halves of one (head, kt) exp tile.
# 'a' = Act accurate exp, 'd' = DVE Schraudolph.  GPSIMD cannot access
# PSUM on TRN2, so the exp wall is carried by Act+DVE alone; the cycle
# of 9 gives Act 20 / DVE 16 halves per 9-kt pair loop.
_EXP_TABLE = [("a", "d"), ("d", "a"), ("a", "d"),
              ("d", "a"), ("a", "d"), ("d", "a"),
              ("a", "d"), ("d", "a"), ("a", "a")]


def _exp_engines(kt, hi, last=False):
    if last:
        return ("a", "d") if hi == 0 else ("d", "a")
    return _EXP_TABLE[(kt * 2 + hi) % 9]


def build_kernel(KP):
    KT = KP // 128
    nc = bacc.Bacc("TRN2")
    ident = nc.dram_tensor("ident", [128, 128], F16, kind="ExternalInput")
    xT = nc.dram_tensor("xT", [H, S], F16, kind="ExternalInput")
    xpT = nc.dram_tensor("xpT", [H, KP], F16, kind="ExternalInput")
    wqT = nc.dram_tensor("wqT", [H, QD], F16, kind="ExternalInput")
    wkT = nc.dram_tensor("wkT", [H, QD], F16, kind="ExternalInput")
    wvT = nc.dram_tensor("wvT", [H, QD], F16, kind="ExternalInput")
    woT = nc.dram_tensor("woT", [QD, H], F16, kind="ExternalInput")
    bk = nc.dram_tensor("bk", [128, KT], F32, kind="ExternalInput")
    bk2 = nc.dram_tensor("bk2", [128, KT], F32, kind="ExternalInput")
    outT = nc.dram_tensor("outT", [H, S], F16, kind="ExternalOutput")

    with tile.TileContext(nc) as tc:
        with tile.TileContext.tile_pool(tc, name="wts", bufs=1) as wp:
            wq_sb = wp.tile([128, HT, QD], F16)
            wk_sb = wp.tile([128, HT, QD], F16)
            wv_sb = wp.tile([128, HT, QD], F16)
            wo_sb = wp.tile([128, MT, H], F16)
            bk_sb = wp.tile([128, KT], F32)
            bk2_sb = wp.tile([128, KT], F32)
            xT_sb = wp.tile([128, HT, S], F16)
            xpT_sb = wp.tile([128, HT, KP], F16)
            qT_sb = wp.tile([128, MT, S], F16)
            kT_sb = wp.tile([128, MT, KP], F16)
            va_sb = wp.tile([128, KT, NHL, 65], F16)   # V rows + ones col
            aT_sb = wp.tile([128, MT, S], F16)
            id_sb = wp.tile([128, 128], F16)

            # --- input DMA on two queues: wk slices + small tensors on the
            # scalar queue, the bulk stream (xpT, wv, wq, xT, wo) on sync.
            # The two queues land wk[ht0] and xpT[ht0] in parallel so the
            # K projection's first matmul starts ~3.7us in and chases the
            # per-ht stream.
            nc.sync.dma_start(out=wk_sb,
                              in_=wkT.ap().rearrange("(t p) d -> p t d",
                                                     p=128))
            for ht in range(HT):
                nc.sync.dma_start(out=xpT_sb[:, ht, :],
                                  in_=xpT.ap()[ht * 128:(ht + 1) * 128, :])
                if ht == 5:
                    nc.sync.dma_start(
                        out=wv_sb,
                        in_=wvT.ap().rearrange("(t p) d -> p t d", p=128))
            nc.scalar.dma_start(out=bk_sb, in_=bk.ap())
            nc.scalar.dma_start(out=bk2_sb, in_=bk2.ap())
            nc.scalar.dma_start(out=id_sb, in_=ident.ap())
            nc.sync.dma_start(out=wq_sb,
                              in_=wqT.ap().rearrange("(t p) d -> p t d",
                                                     p=128))
            for hp in range(HT // 2):
                nc.sync.dma_start(
                    out=xT_sb[:, 2 * hp:2 * hp + 2, :],
                    in_=xT.ap()[hp * 256:(hp + 1) * 256, :].rearrange(
                        "(t p) s -> p t s", p=128))
            nc.sync.dma_start(out=wo_sb,
                              in_=woT.ap().rearrange("(t p) d -> p t d",
                                                     p=128))
            nc.vector.memset(va_sb[:, :, :, 64:65], 1.0)

            wu_sb = wp.tile([128, 128], F16)
            nc.vector.memset(wu_sb, 0.0)

            evac_flip = [0]

            def evac(dst, src):
                # alternate psum evacuations between Act and DVE
                if evac_flip[0] % 2 == 0:
                    nc.scalar.copy(dst, src)
                else:
                    nc.vector.tensor_copy(dst, src)
                evac_flip[0] += 1

            # single PSUM pool, 8 banks: tag "pss" = 5 rotating [128,512]f32
            # banks (projections / scores / out-proj / PE-transpose spill),
            # tag "pav" = 2 banks (AV accumulators, K remainder chunks, V
            # projection), tag "pdn" = 1 bank (softmax denominators).
            with tile.TileContext.tile_pool(tc, name="pss", bufs=5,
                                            space="PSUM") as pss:
                def pstile(shape, tag, bufs, name, dt=F32):
                    return pss.tile(shape, dt, tag=tag, bufs=bufs, name=name)

                if True:
                    # ---- PE warmup: tiny matmuls on zeros keep the tensor
                    # engine's p-state ramp running until the first real
                    # matmul's inputs land (~4.5us), so the K projection
                    # starts at full clock.
                    wps = pstile([128, 128], "pav", 2, "ps_wu")
                    for _ in range(40):
                        nc.tensor.matmul(wps, wu_sb, wu_sb,
                                         start=True, stop=True)

                    # ---- K^T projection, ht-outer so matmuls chase the DMA.
                    kchunks = []
                    for mt in range(MT):
                        for po, pw in _chunks(KP, 512):
                            if pw > 128:
                                ps = pstile([128, 512], "pss", 5,
                                            f"ps_k{mt}_{po}")
                            else:
                                ps = pstile([128, 128], "pav", 2,
                                            f"ps_k{mt}_{po}")
                            kchunks.append((mt, po, pw, ps))
                    # the first two V-projection key-tiles ride along in
                    # the K ht-loop so the PE outpaces the xpT DMA stream
                    vps = [pstile([128, QD], "pav", 2, f"ps_v{st}")
                           for st in range(2)]
                    for ht in range(HT):
                        for mt, po, pw, ps in kchunks:
                            nc.tensor.matmul(
                                ps[:, 0:pw],
                                wk_sb[:, ht, mt * 128:(mt + 1) * 128],
                                xpT_sb[:, ht, po:po + pw],
                                start=(ht == 0), stop=(ht == HT - 1))
                        for st in range(2):
                            nc.tensor.matmul(
                                vps[st],
                                xpT_sb[:, ht, st * 128:(st + 1) * 128],
                                wv_sb[:, ht, :],
                                start=(ht == 0), stop=(ht == HT - 1))
                    for mt, po, pw, ps in sorted(kchunks,
                                                 key=lambda c: -c[1]):
                        evac(kT_sb[:, mt, po:po + pw], ps[:, 0:pw])
                    for st in range(2):
                        evac(va_sb[:, st, :, 0:64],
                             vps[st].rearrange("p (h d) -> p h d", h=NHL))

                    # ---- V projection, remaining key-tiles
                    for st in range(2, KT):
                        pv = pstile([128, QD], "pav", 2, "ps_v")
                        for ht in range(HT):
                            nc.tensor.matmul(
                                pv, xpT_sb[:, ht, st * 128:(st + 1) * 128],
                                wv_sb[:, ht, :],
                                start=(ht == 0), stop=(ht == HT - 1))
                        evac(va_sb[:, st, :, 0:64],
                             pv.rearrange("p (h d) -> p h d", h=NHL))

                # ---- Q^T projection for a query half
                def emit_q(po, mts=(0, 1)):
                    for mt in mts:
                        for co, cw in _chunks(1024, 512):
                            ps = pstile([128, 512], "pss", 5,
                                        f"ps_q{mt}_{po + co}")
                            for ht in range(HT):
                                nc.tensor.matmul(
                                    ps,
                                    wq_sb[:, ht, mt * 128:(mt + 1) * 128],
                                    xT_sb[:, ht, po + co:po + co + cw],
                                    start=(ht == 0), stop=(ht == HT - 1))
                            evac(qT_sb[:, mt, po + co:po + co + cw], ps)

                emit_q(0)

                # ---- attention + interleaved out-projection phases
                with tile.TileContext.tile_pool(tc, name="pex", bufs=12) as pxp, \
                     tile.TileContext.tile_pool(tc, name="an", bufs=3) as anp, \
                     tile.TileContext.tile_pool(tc, name="rc", bufs=4) as rcp, \
                     tile.TileContext.tile_pool(tc, name="stg", bufs=10) as sgp:

                    def attn_pair(pair, po):
                        """Attention for heads `pair` on queries
                        [po*1024, (po+1)*1024)."""
                        mtq = pair[0] // 2
                        q0 = po * 1024
                        pav = {}
                        pend = {h: [] for h in pair}
                        for h in pair:
                            pav[h] = pstile([128, QT, 64], "pav", 2,
                                            f"pav{h}_{po}")
                        # softmax denominators for both heads (64B bank)
                        pden = pstile([128, 2, QT], "pdn", 1,
                                      f"pdn{pair[0]}_{po}")

                        def flush_av(hi, h):
                            # start=True zeroes the whole 2KB PSUM bank, so
                            # only the very first matmul into each bank may
                            # carry it; the other interleaved accumulation
                            # groups land on pending-zero bytes (zeroed on
                            # first write).
                            pkt, ppx = pend[h].pop(0)
                            for qt in range(QT):
                                pxs = ppx[:, qt * 128:(qt + 1) * 128]
                                nc.tensor.matmul(
                                    pav[h][:, qt, :], pxs,
                                    va_sb[:, pkt, h, 0:64],
                                    start=(pkt == 0 and qt == 0),
                                    stop=(pkt == KT - 1 and qt == QT - 1),
                                    skip_group_check=True)
                                nc.tensor.matmul(
                                    pden[:, hi, qt:qt + 1], pxs,
                                    va_sb[:, pkt, h, 64:65],
                                    start=(pkt == 0 and qt == 0 and hi == 0),
                                    stop=(pkt == KT - 1 and qt == QT - 1
                                          and hi == 1),
                                    skip_group_check=True)

                        for kt in range(KT):
                            for hi, h in enumerate(pair):
                                rb = (h * HD) % 128
                                px = pxp.tile([128, 1024], F16, tag="pex",
                                              name=f"pex{h}_{po}_{kt}")
                                engs = _exp_engines(kt, hi, last=(kt == KT - 1))
                                for ci, (co, cw) in enumerate(_chunks(1024, 512)):
                                    ps = pstile([128, 512], "pss", 5,
                                                f"ps_s{h}_{po}_{kt}_{co}")
                                    nc.tensor.matmul(
                                        ps,
                                        kT_sb[rb:rb + HD, mtq,
                                              kt * 128:(kt + 1) * 128],
                                        qT_sb[rb:rb + HD, mtq,
                                              q0 + co:q0 + co + cw],
                                        start=True, stop=True)
                                    eng = engs[ci]
                                    if eng == "a":
                                        nc.scalar.activation(
                                            out=px[:, co:co + cw],
                                            in_=ps,
                                            func=mybir.ActivationFunctionType.Exp,
                                            bias=bk_sb[:, kt:kt + 1], scale=1.0)
                                    else:
                                        nc.vector.tensor_scalar(
                                            px[:, co:co + cw].bitcast(U16),
                                            ps,
                                            AEXP, bk2_sb[:, kt:kt + 1],
                                            mybir.AluOpType.mult,
                                            mybir.AluOpType.add)
                                pend[h].append((kt, px))
                            # drain the AV backlog harder near the end of the
                            # kt loop so normalize/transpose start promptly
                            max_pend = 2 if kt < KT - 2 else (KT - 1 - kt)
                            for hi, h in enumerate(pair):
                                while len(pend[h]) > max_pend:
                                    flush_av(hi, h)
                        an = anp.tile([128, QT, 128], F16, tag="an",
                                      name=f"an{pair[0]}_{po}")
                        rcb = {}
                        for hi, h in enumerate(pair):
                            while pend[h]:
                                flush_av(hi, h)
                            rc = rcp.tile([128, QT], F32, tag="rc",
                                          name=f"rc{h}_{po}")
                            nc.vector.reciprocal(rc, pden[:, hi, :])
                            r = rc.rearrange("p (q o) -> p q o", o=1)
                            rcb[h] = r.broadcast_to([128, QT, 64])
                        # normalize a_n[q, qt, d] = pav[q, qt, d] / den in
                        # qt-half groups so transposes can start early
                        for hr in (0, QT // 2):
                            for hi, h in enumerate(pair):
                                nc.vector.tensor_tensor(
                                    out=an[:, hr:hr + QT // 2,
                                           hi * 64:hi * 64 + 64],
                                    in0=pav[h][:, hr:hr + QT // 2, :],
                                    in1=rcb[h][:, hr:hr + QT // 2, :],
                                    op=mybir.AluOpType.mult)

                        # transpose [q, d] -> [d, q].  The first pair rides
                        # the DMA XBAR (latency-tolerant: a full pair of
                        # compute follows); the last pair, which gates the
                        # out-projection, uses the PE array + tiny evacs.
                        # Emission is deferred to the returned closure so the
                        # caller can slot PE filler before the PE transposes.
                        tr_tile = []

                        def finish_half(hq):
                            qts = range(hq * (QT // 2), (hq + 1) * (QT // 2))
                            if mtq == 0:
                                for qt in qts:
                                    nc.sync.dma_start_transpose(
                                        aT_sb[:, mtq,
                                              q0 + qt * 128:q0 + (qt + 1) * 128],
                                        an[:, qt, :])
                                return
                            if not tr_tile:
                                tr_tile.append(pstile([128, QT, 128], "pav", 2,
                                                      f"tr{po}", dt=F16))
                            tr = tr_tile[0]
                            for qt in qts:
                                # sub-bank writes: only the first transpose
                                # into the tr bank may carry start=True
                                nc.tensor.matmul(
                                    tr[:, qt, :], an[:, qt, :], id_sb,
                                    is_transpose=True,
                                    start=(hq == 0 and qt == qts[0]),
                                    stop=(hq == 1 and qt == qts[-1]),
                                    skip_group_check=True)
                            engs = ((nc.scalar, nc.vector, nc.scalar,
                                     nc.vector) if hq == 0 else
                                    (nc.vector, nc.scalar, nc.vector,
                                     nc.scalar))
                            for qt in qts:
                                dst = aT_sb[:, mtq,
                                            q0 + qt * 128:q0 + (qt + 1) * 128]
                                copy_half(engs[qt % 4], dst, tr[:, qt, :])

                        def finish():
                            finish_half(0)
                            finish_half(1)

                        finish.half = finish_half
                        return finish

                    def copy_half(eng, dst, src):
                        if eng is nc.scalar:
                            eng.copy(dst, src)
                        else:
                            eng.tensor_copy(dst, src)

                    def out_proj(ho, jts, eoff=0):
                        """Out-projection rows jts, columns [ho*1024, +1024).
                        PSUM evacuation runs as 512-halves on two engines so
                        the pss slots free at PE pace."""
                        q0 = ho * 1024
                        for ji, jt in enumerate(jts):
                            stg = sgp.tile([128, 1024], F16, tag="stg",
                                           name="stage")
                            e01 = ((nc.scalar, nc.vector)
                                   if (ji + eoff) % 2 == 0
                                   else (nc.vector, nc.scalar))
                            for ci, (co, cw) in enumerate(_chunks(1024, 512)):
                                pf = pstile([128, 512], "pss", 5,
                                            f"ps_f{jt}_{ho}_{co}")
                                for mt in range(MT):
                                    nc.tensor.matmul(
                                        pf,
                                        wo_sb[:, mt, jt * 128:(jt + 1) * 128],
                                        aT_sb[:, mt, q0 + co:q0 + co + cw],
                                        start=(mt == 0), stop=(mt == MT - 1))
                                copy_half(e01[ci], stg[:, co:co + cw], pf)
                            nc.sync.dma_start(
                                out=outT.ap()[jt * 128:(jt + 1) * 128,
                                              q0:q0 + 1024],
                                in_=stg)

                    attn_pair((0, 1), 0)()
                    f2 = attn_pair((2, 3), 0)
                    emit_q(1024, mts=(0,))   # PE filler while normalize lands
                    f2()
                    emit_q(1024, mts=(1,))
                    out_proj(0, range(5))
                    attn_pair((0, 1), 1)()
                    out_proj(0, range(5, 6), eoff=1)   # fills pair transition
                    f4 = attn_pair((2, 3), 1)
                    # held-back ho=0 rows fill the PE while the last pair's
                    # normalize lands, then its PE transposes + evacs run
                    out_proj(0, range(6, 7), eoff=1)
                    f4.half(0)
                    out_proj(0, range(7, HT), eoff=1)
                    f4.half(1)
                    out_proj(1, range(HT))

    nc.compile()
    return nc


def _prep_inputs(hidden_states, attention_mask, w_qkv, w_out):
    """Shard + transpose + quantize inputs for the 8 cores."""
    hs = np.asarray(hidden_states, dtype=np.float32)
    mask = np.asarray(attention_mask)
    wqkv = np.asarray(w_qkv, dtype=np.float32)
    wo = np.asarray(w_out, dtype=np.float32)

    idxs = [np.nonzero(mask[b] != 0)[0] for b in range(B)]
    counts = [len(ix) for ix in idxs]
    KP = max(128, ((max(counts) + 127) // 128) * 128)
    KT = KP // 128

    xTs, xpTs, bks, bk2s = [], [], [], []
    for b in range(B):
        xb = hs[b].astype(NPF16)
        xTs.append(np.ascontiguousarray(xb.T))
        xp = np.zeros((KP, H), dtype=NPF16)
        xp[:counts[b]] = xb[idxs[b]]
        xpTs.append(np.ascontiguousarray(xp.T))
        bias = np.zeros(KP, dtype=np.float32)
        bias[counts[b]:] = -30000.0
        bias = np.ascontiguousarray(bias.reshape(KT, 128).T)
        bks.append(bias)
        bk2s.append(np.ascontiguousarray(
            (bias * AEXP + BEXP).astype(np.float32)))

    ident = np.ascontiguousarray(np.eye(128, dtype=NPF16))
    in_maps = []
    for c in range(NCORES):
        b, hb = c // CPB, c % CPB
        sl = slice(hb * QD, (hb + 1) * QD)
        in_maps.append({
            "ident": ident,
            "xT": xTs[b],
            "xpT": xpTs[b],
            "wqT": np.ascontiguousarray(
                (wqkv[sl, :] * SCALE).astype(NPF16).T),
            "wkT": np.ascontiguousarray(
                wqkv[H + sl.start:H + sl.stop, :].astype(NPF16).T),
            "wvT": np.ascontiguousarray(
                wqkv[2 * H + sl.start:2 * H + sl.stop, :].astype(NPF16).T),
            "woT": np.ascontiguousarray(wo[:, sl].astype(NPF16).T),
            "bk": bks[b],
            "bk2": bk2s[b],
        })
    return KP, in_maps


_NC_CACHE = {}


def kernel(hidden_states, attention_mask, w_qkv, w_out):
    KP, in_maps = _prep_inputs(hidden_states, attention_mask, w_qkv, w_out)
    if KP not in _NC_CACHE:
        _NC_CACHE[KP] = build_kernel(KP)
    nc = _NC_CACHE[KP]
    res = bass_utils.run_bass_kernel_spmd(nc, in_maps,
                                          core_ids=list(range(NCORES)))
    out = np.empty((B, S, H), dtype=np.float32)
    for b in range(B):
        acc = res.results[b * CPB]["outT"].astype(np.float32).copy()
        for c in range(b * CPB + 1, (b + 1) * CPB):
            acc += res.results[c]["outT"]
        out[b] = acc.T
    return out


# revision 37
# speedup vs baseline: 1.2617x; 1.0094x over previous
"""Bass/Trainium2 kernel for nn_BaseAttention (B=2, S=2048, H=1024, NH=16, HD=64).

Sharding: 8 cores = 2 batches x 4 head-groups (4 heads each core).
Each core computes, for its (batch b, head-group hb):
    qkv slice -> attention over packed masked keys -> partial out-projection
and writes partial^T [H, S].  Host sums the 4 partials per batch and
transposes.

v4 design (all-fp16 data path; cost-model-guided):
  * fp16 everywhere instead of bf16: same 1 cyc/row matmul throughput and
    identical DMA bytes, but 8x less quantization error -- the error budget
    is then dominated by the Schraudolph exp tiles alone.
  * AV computed in [q, d] orientation: stationary = exp'd score tile
    [128 keys, 128 q], moving = V-augmented [128 keys, 65] (65th column of
    ones gives the softmax denominator).  Cost-model matmul time is
    out_free x 1 cyc, so AV drops ~2x vs the [d^T, q] orientation.
    Normalization becomes a per-partition tensor op (reciprocal of the
    denominator column + broadcast multiply), and the [q, d] -> [d, q]
    transpose needed by the out-projection runs on the DMA XBAR
    (dma_start_transpose), costing no PE/ACT/DVE time and no PSUM banks.
  * Scores in S^T layout [key_part, q_free], exp split per 512-half across
    Act (accurate exp) / DVE / Pool (Schraudolph: uint16(round(s*A + B))
    bits ARE fp16(exp(s)); saturates to +0 for masked keys).  Splitting
    halves keeps the exp service latency under the scores-psum reuse window.
  * Out-projection for queries 0:1024 runs between the two attention halves
    so its output DMA overlaps the second half's compute; Q-projection for
    queries 1024:2048 is PE filler in the same slot.
  * Masked keys packed on host (KP = ceil(max_count/128)*128); 1/sqrt(HD)
    folded into wq on the host; key-padding bias fused into exp.
"""

import numpy as np

import concourse.bass as bass
import concourse.mybir as mybir
import concourse.tile as tile
from concourse import bacc
from concourse import bass_utils

B, S, H = 2, 2048, 1024
NH, HD = 16, 64
SCALE = HD ** -0.5
NCORES = 8
CPB = NCORES // B          # cores per batch = 4
NHL = NH // CPB            # local heads per core = 4
QD = NHL * HD              # local head-dim total = 256
HT = H // 128              # k-tiles over hidden dim = 8
MT = QD // 128             # partition-tiles over local head dims = 2
PO = 2                     # query halves (1024 each)
QT = 8                     # 128-query tiles per half

F32 = mybir.dt.float32
F16 = mybir.dt.float16
U16 = mybir.dt.uint16
NPF16 = np.float16

# Schraudolph exp -> fp16 bits: u16 = round(s * AEXP + BEXP), saturating at
# 0.  AEXP = 2^10/ln2 (fp16 exponent LSB is bit 10); the -60 fraction-bias
# offset minimizes the max relative error over the sawtooth.
AEXP = 1024.0 / float(np.log(2.0))
BEXP = 15.0 * 1024.0 - 60.0


def _chunks(total, size):
    out = []
    o = 0
    while o < total:
        c = min(size, total - o)
        out.append((o, c))
        o += c
    return out


# Engines for the two 512-halves of one (head, kt) exp tile.
# 'a' = Act accurate exp, 'd' = DVE Schraudolph.  GPSIMD cannot access
# PSUM on TRN2, so the exp wall is carried by Act+DVE alone; the cycle
# of 9 gives Act 20 / DVE 16 halves per 9-kt pair loop.
_EXP_TABLE = [("a", "d"), ("d", "a"), ("a", "d"),
              ("d", "a"), ("a", "d"), ("d", "a"),
              ("a", "d"), ("d", "a"), ("a", "a")]


def _exp_engines(kt, hi, last=False):
    if last:
        return ("a", "d") if hi == 0 else ("d", "a")
    return _EXP_TABLE[(kt * 2 + hi) % 9]


def build_kernel(KP):
    KT = KP // 128
    nc = bacc.Bacc("TRN2")
    ident = nc.dram_tensor("ident", [128, 128], F16, kind="ExternalInput")
    xT = nc.dram_tensor("xT", [H, S], F16, kind="ExternalInput")
    xpT = nc.dram_tensor("xpT", [H, KP], F16, kind="ExternalInput")
    wqT = nc.dram_tensor("wqT", [H, QD], F16, kind="ExternalInput")
    wkT = nc.dram_tensor("wkT", [H, QD], F16, kind="ExternalInput")
    wvT = nc.dram_tensor("wvT", [H, QD], F16, kind="ExternalInput")
    woT = nc.dram_tensor("woT", [QD, H], F16, kind="ExternalInput")
    bk = nc.dram_tensor("bk", [128, KT], F32, kind="ExternalInput")
    bk2 = nc.dram_tensor("bk2", [128, KT], F32, kind="ExternalInput")
    outT = nc.dram_tensor("outT", [H, S], F16, kind="ExternalOutput")

    with tile.TileContext(nc) as tc:
        with tile.TileContext.tile_pool(tc, name="wts", bufs=1) as wp:
            wq_sb = wp.tile([128, HT, QD], F16)
            wk_sb = wp.tile([128, HT, QD], F16)
            wv_sb = wp.tile([128, HT, QD], F16)
            wo_sb = wp.tile([128, MT, H], F16)
            bk_sb = wp.tile([128, KT], F32)
            bk2_sb = wp.tile([128, KT], F32)
            xT_sb = wp.tile([128, HT, S], F16)
            xpT_sb = wp.tile([128, HT, KP], F16)
            qT_sb = wp.tile([128, MT, S], F16)
            kT_sb = wp.tile([128, MT, KP], F16)
            va_sb = wp.tile([128, KT, NHL, 65], F16)   # V rows + ones col
            aT_sb = wp.tile([128, MT, S], F16)
            id_sb = wp.tile([128, 128], F16)

            wu_sb = wp.tile([128, 128], F16)
            nc.vector.memset(wu_sb, 0.0)

            # --- input DMA on two queues: wk slices + small tensors on the
            # scalar queue, the bulk stream (xpT, wv, wq, xT, wo) on sync.
            # The two queues land wk[ht0] and xpT[ht0] in parallel so the
            # K projection's first matmul starts ~3.7us in and chases the
            # per-ht stream.
            nc.sync.dma_start(out=wk_sb,
                              in_=wkT.ap().rearrange("(t p) d -> p t d",
                                                     p=128))
            for ht in range(HT):
                nc.sync.dma_start(out=xpT_sb[:, ht, :],
                                  in_=xpT.ap()[ht * 128:(ht + 1) * 128, :])
                if ht == 5:
                    nc.sync.dma_start(
                        out=wv_sb,
                        in_=wvT.ap().rearrange("(t p) d -> p t d", p=128))
            nc.scalar.dma_start(out=bk_sb, in_=bk.ap())
            nc.scalar.dma_start(out=bk2_sb, in_=bk2.ap())
            nc.scalar.dma_start(out=id_sb, in_=ident.ap())
            nc.sync.dma_start(out=wq_sb,
                              in_=wqT.ap().rearrange("(t p) d -> p t d",
                                                     p=128))
            for hp in range(HT // 2):
                nc.sync.dma_start(
                    out=xT_sb[:, 2 * hp:2 * hp + 2, :],
                    in_=xT.ap()[hp * 256:(hp + 1) * 256, :].rearrange(
                        "(t p) s -> p t s", p=128))
            nc.sync.dma_start(out=wo_sb,
                              in_=woT.ap().rearrange("(t p) d -> p t d",
                                                     p=128))
            nc.vector.memset(va_sb[:, :, :, 64:65], 1.0)

            evac_flip = [0]

            def evac(dst, src):
                # alternate psum evacuations between Act and DVE
                if evac_flip[0] % 2 == 0:
                    nc.scalar.copy(dst, src)
                else:
                    nc.vector.tensor_copy(dst, src)
                evac_flip[0] += 1

            # single PSUM pool, 8 banks: tag "pss" = 5 rotating [128,512]f32
            # banks (projections / scores / out-proj / PE-transpose spill),
            # tag "pav" = 2 banks (AV accumulators, K remainder chunks, V
            # projection), tag "pdn" = 1 bank (softmax denominators).
            with tile.TileContext.tile_pool(tc, name="pss", bufs=5,
                                            space="PSUM") as pss:
                def pstile(shape, tag, bufs, name, dt=F32):
                    return pss.tile(shape, dt, tag=tag, bufs=bufs, name=name)

                if True:
                    # ---- PE warmup: tiny matmuls on zeros keep the tensor
                    # engine's p-state ramp running until the first real
                    # matmul's inputs land (~4.5us), so the K projection
                    # starts at full clock.
                    wps = pstile([128, 512], "pss", 5, "ps_wu")
                    for _ in range(40):
                        nc.tensor.matmul(wps[:, 0:128], wu_sb, wu_sb,
                                         start=True, stop=True)

                    # ---- K^T projection, ht-outer so matmuls chase the DMA.
                    kchunks = []
                    for mt in range(MT):
                        for po, pw in _chunks(KP, 512):
                            if pw > 128:
                                ps = pstile([128, 512], "pss", 5,
                                            f"ps_k{mt}_{po}")
                            else:
                                ps = pstile([128, 128], "pav", 2,
                                            f"ps_k{mt}_{po}")
                            kchunks.append((mt, po, pw, ps))
                    # the first two V-projection key-tiles ride along in
                    # the K ht-loop so the PE outpaces the xpT DMA stream
                    vps = [pstile([128, QD], "pss", 5, "ps_v0"),
                           pstile([128, QD], "pdn", 1, "ps_v1")]
                    for ht in range(HT):
                        for mt, po, pw, ps in kchunks:
                            nc.tensor.matmul(
                                ps[:, 0:pw],
                                wk_sb[:, ht, mt * 128:(mt + 1) * 128],
                                xpT_sb[:, ht, po:po + pw],
                                start=(ht == 0), stop=(ht == HT - 1))
                        for st in range(2):
                            nc.tensor.matmul(
                                vps[st],
                                xpT_sb[:, ht, st * 128:(st + 1) * 128],
                                wv_sb[:, ht, :],
                                start=(ht == 0), stop=(ht == HT - 1))
                    for mt, po, pw, ps in sorted(kchunks,
                                                 key=lambda c: -c[1]):
                        evac(kT_sb[:, mt, po:po + pw], ps[:, 0:pw])
                    for st in range(2):
                        evac(va_sb[:, st, :, 0:64],
                             vps[st].rearrange("p (h d) -> p h d", h=NHL))

                    # ---- V projection, remaining key-tiles
                    for st in range(2, KT):
                        pv = pstile([128, QD], "pav", 2, "ps_v")
                        for ht in range(HT):
                            nc.tensor.matmul(
                                pv, xpT_sb[:, ht, st * 128:(st + 1) * 128],
                                wv_sb[:, ht, :],
                                start=(ht == 0), stop=(ht == HT - 1))
                        evac(va_sb[:, st, :, 0:64],
                             pv.rearrange("p (h d) -> p h d", h=NHL))

                # ---- Q^T projection for a query half
                def emit_q(po, mts=(0, 1)):
                    for mt in mts:
                        for co, cw in _chunks(1024, 512):
                            ps = pstile([128, 512], "pss", 5,
                                        f"ps_q{mt}_{po + co}")
                            for ht in range(HT):
                                nc.tensor.matmul(
                                    ps,
                                    wq_sb[:, ht, mt * 128:(mt + 1) * 128],
                                    xT_sb[:, ht, po + co:po + co + cw],
                                    start=(ht == 0), stop=(ht == HT - 1))
                            evac(qT_sb[:, mt, po + co:po + co + cw], ps)

                emit_q(0)

                # ---- attention + interleaved out-projection phases
                with tile.TileContext.tile_pool(tc, name="pex", bufs=12) as pxp, \
                     tile.TileContext.tile_pool(tc, name="an", bufs=3) as anp, \
                     tile.TileContext.tile_pool(tc, name="rc", bufs=4) as rcp, \
                     tile.TileContext.tile_pool(tc, name="stg", bufs=10) as sgp:

                    def attn_setup(pair, po):
                        """Attention state for heads `pair` on queries
                        [po*1024, (po+1)*1024)."""
                        s = {"pair": pair, "po": po, "mtq": pair[0] // 2,
                             "q0": po * 1024, "pend": {h: [] for h in pair}}
                        s["pav"] = {h: pstile([128, QT, 64], "pav", 2,
                                              f"pav{h}_{po}") for h in pair}
                        # softmax denominators for both heads (64B bank)
                        s["pden"] = pstile([128, 2, QT], "pdn", 1,
                                           f"pdn{pair[0]}_{po}")
                        return s

                    def attn_kts(s, kts):
                        pair, po, q0 = s["pair"], s["po"], s["q0"]
                        mtq, pav, pden = s["mtq"], s["pav"], s["pend"]
                        pav, pden = s["pav"], s["pden"]
                        pend = s["pend"]

                        def flush_av(hi, h):
                            # start=True zeroes the whole 2KB PSUM bank, so
                            # only the very first matmul into each bank may
                            # carry it; the other interleaved accumulation
                            # groups land on pending-zero bytes (zeroed on
                            # first write).
                            pkt, ppx = pend[h].pop(0)
                            for qt in range(QT):
                                pxs = ppx[:, qt * 128:(qt + 1) * 128]
                                nc.tensor.matmul(
                                    pav[h][:, qt, :], pxs,
                                    va_sb[:, pkt, h, 0:64],
                                    start=(pkt == 0 and qt == 0),
                                    stop=(pkt == KT - 1 and qt == QT - 1),
                                    skip_group_check=True)
                                nc.tensor.matmul(
                                    pden[:, hi, qt:qt + 1], pxs,
                                    va_sb[:, pkt, h, 64:65],
                                    start=(pkt == 0 and qt == 0 and hi == 0),
                                    stop=(pkt == KT - 1 and qt == QT - 1
                                          and hi == 1),
                                    skip_group_check=True)

                        for kt in range(KT):
                            for hi, h in enumerate(pair):
                                rb = (h * HD) % 128
                                px = pxp.tile([128, 1024], F16, tag="pex",
                                              name=f"pex{h}_{po}_{kt}")
                                engs = _exp_engines(kt, hi, last=(kt == KT - 1))
                                for ci, (co, cw) in enumerate(_chunks(1024, 512)):
                                    ps = pstile([128, 512], "pss", 5,
                                                f"ps_s{h}_{po}_{kt}_{co}")
                                    nc.tensor.matmul(
                                        ps,
                                        kT_sb[rb:rb + HD, mtq,
                                              kt * 128:(kt + 1) * 128],
                                        qT_sb[rb:rb + HD, mtq,
                                              q0 + co:q0 + co + cw],
                                        start=True, stop=True)
                                    eng = engs[ci]
                                    if eng == "a":
                                        nc.scalar.activation(
                                            out=px[:, co:co + cw],
                                            in_=ps,
                                            func=mybir.ActivationFunctionType.Exp,
                                            bias=bk_sb[:, kt:kt + 1], scale=1.0)
                                    else:
                                        nc.vector.tensor_scalar(
                                            px[:, co:co + cw].bitcast(U16),
                                            ps,
                                            AEXP, bk2_sb[:, kt:kt + 1],
                                            mybir.AluOpType.mult,
                                            mybir.AluOpType.add)
                                pend[h].append((kt, px))
                            # drain the AV backlog harder near the end of the
                            # kt loop so normalize/transpose start promptly
                            max_pend = 2 if kt < KT - 2 else (KT - 1 - kt)
                            for hi, h in enumerate(pair):
                                while len(pend[h]) > max_pend:
                                    flush_av(hi, h)
                        an = anp.tile([128, QT, 128], F16, tag="an",
                                      name=f"an{pair[0]}_{po}")
                        rcb = {}
                        for hi, h in enumerate(pair):
                            while pend[h]:
                                flush_av(hi, h)
                            rc = rcp.tile([128, QT], F32, tag="rc",
                                          name=f"rc{h}_{po}")
                            nc.vector.reciprocal(rc, pden[:, hi, :])
                            r = rc.rearrange("p (q o) -> p q o", o=1)
                            rcb[h] = r.broadcast_to([128, QT, 64])
                        # normalize a_n[q, qt, d] = pav[q, qt, d] / den in
                        # qt-half groups so transposes can start early
                        for hr in (0, QT // 2):
                            for hi, h in enumerate(pair):
                                nc.vector.tensor_tensor(
                                    out=an[:, hr:hr + QT // 2,
                                           hi * 64:hi * 64 + 64],
                                    in0=pav[h][:, hr:hr + QT // 2, :],
                                    in1=rcb[h][:, hr:hr + QT // 2, :],
                                    op=mybir.AluOpType.mult)

                        # transpose [q, d] -> [d, q].  The first pair rides
                        # the DMA XBAR (latency-tolerant: a full pair of
                        # compute follows); the last pair, which gates the
                        # out-projection, uses the PE array + tiny evacs.
                        # Emission is deferred to the returned closure so the
                        # caller can slot PE filler before the PE transposes.
                        tr_tile = []

                        def finish_half(hq):
                            qts = range(hq * (QT // 2), (hq + 1) * (QT // 2))
                            if mtq == 0:
                                for qt in qts:
                                    nc.sync.dma_start_transpose(
                                        aT_sb[:, mtq,
                                              q0 + qt * 128:q0 + (qt + 1) * 128],
                                        an[:, qt, :])
                                return
                            if not tr_tile:
                                tr_tile.append(pstile([128, QT, 128], "pav", 2,
                                                      f"tr{po}", dt=F16))
                            tr = tr_tile[0]
                            for qt in qts:
                                # sub-bank writes: only the first transpose
                                # into the tr bank may carry start=True
                                nc.tensor.matmul(
                                    tr[:, qt, :], an[:, qt, :], id_sb,
                                    is_transpose=True,
                                    start=(hq == 0 and qt == qts[0]),
                                    stop=(hq == 1 and qt == qts[-1]),
                                    skip_group_check=True)
                            engs = ((nc.scalar, nc.vector, nc.scalar,
                                     nc.vector) if hq == 0 else
                                    (nc.vector, nc.scalar, nc.vector,
                                     nc.scalar))
                            for qt in qts:
                                dst = aT_sb[:, mtq,
                                            q0 + qt * 128:q0 + (qt + 1) * 128]
                                copy_half(engs[qt % 4], dst, tr[:, qt, :])

                        def finish():
                            finish_half(0)
                            finish_half(1)

                        finish.half = finish_half
                        return finish

                    def copy_half(eng, dst, src):
                        if eng is nc.scalar:
                            eng.copy(dst, src)
                        else:
                            eng.tensor_copy(dst, src)

                    def out_proj(ho, jts, eoff=0):
                        """Out-projection rows jts, columns [ho*1024, +1024).
                        PSUM evacuation runs as 512-halves on two engines so
                        the pss slots free at PE pace."""
                        q0 = ho * 1024
                        for ji, jt in enumerate(jts):
                            stg = sgp.tile([128, 1024], F16, tag="stg",
                                           name="stage")
                            e01 = ((nc.scalar, nc.vector)
                                   if (ji + eoff) % 2 == 0
                                   else (nc.vector, nc.scalar))
                            for ci, (co, cw) in enumerate(_chunks(1024, 512)):
                                pf = pstile([128, 512], "pss", 5,
                                            f"ps_f{jt}_{ho}_{co}")
                                for mt in range(MT):
                                    nc.tensor.matmul(
                                        pf,
                                        wo_sb[:, mt, jt * 128:(jt + 1) * 128],
                                        aT_sb[:, mt, q0 + co:q0 + co + cw],
                                        start=(mt == 0), stop=(mt == MT - 1))
                                copy_half(e01[ci], stg[:, co:co + cw], pf)
                            nc.sync.dma_start(
                                out=outT.ap()[jt * 128:(jt + 1) * 128,
                                              q0:q0 + 1024],
                                in_=stg)

                    attn_pair((0, 1), 0)()
                    f2 = attn_pair((2, 3), 0)
                    emit_q(1024, mts=(0,))   # PE filler while normalize lands
                    f2()
                    emit_q(1024, mts=(1,))
                    out_proj(0, range(5))
                    attn_pair((0, 1), 1)()
                    f4 = attn_pair((2, 3), 1)
                    # held-back ho=0 rows fill the PE while the last pair's
                    # normalize lands, then its PE transposes + evacs run
                    out_proj(0, range(5, 6), eoff=1)
                    f4.half(0)
                    out_proj(0, range(6, HT), eoff=1)
                    f4.half(1)
                    out_proj(1, range(HT))

    nc.compile()
    return nc


def _prep_inputs(hidden_states, attention_mask, w_qkv, w_out):
    """Shard + transpose + quantize inputs for the 8 cores."""
    hs = np.asarray(hidden_states, dtype=np.float32)
    mask = np.asarray(attention_mask)
    wqkv = np.asarray(w_qkv, dtype=np.float32)
    wo = np.asarray(w_out, dtype=np.float32)

    idxs = [np.nonzero(mask[b] != 0)[0] for b in range(B)]
    counts = [len(ix) for ix in idxs]
    KP = max(128, ((max(counts) + 127) // 128) * 128)
    KT = KP // 128

    xTs, xpTs, bks, bk2s = [], [], [], []
    for b in range(B):
        xb = hs[b].astype(NPF16)
        xTs.append(np.ascontiguousarray(xb.T))
        xp = np.zeros((KP, H), dtype=NPF16)
        xp[:counts[b]] = xb[idxs[b]]
        xpTs.append(np.ascontiguousarray(xp.T))
        bias = np.zeros(KP, dtype=np.float32)
        bias[counts[b]:] = -30000.0
        bias = np.ascontiguousarray(bias.reshape(KT, 128).T)
        bks.append(bias)
        bk2s.append(np.ascontiguousarray(
            (bias * AEXP + BEXP).astype(np.float32)))

    ident = np.ascontiguousarray(np.eye(128, dtype=NPF16))
    in_maps = []
    for c in range(NCORES):
        b, hb = c // CPB, c % CPB
        sl = slice(hb * QD, (hb + 1) * QD)
        in_maps.append({
            "ident": ident,
            "xT": xTs[b],
            "xpT": xpTs[b],
            "wqT": np.ascontiguousarray(
                (wqkv[sl, :] * SCALE).astype(NPF16).T),
            "wkT": np.ascontiguousarray(
                wqkv[H + sl.start:H + sl.stop, :].astype(NPF16).T),
            "wvT": np.ascontiguousarray(
                wqkv[2 * H + sl.start:2 * H + sl.stop, :].astype(NPF16).T),
            "woT": np.ascontiguousarray(wo[:, sl].astype(NPF16).T),
            "bk": bks[b],
            "bk2": bk2s[b],
        })
    return KP, in_maps


_NC_CACHE = {}


def kernel(hidden_states, attention_mask, w_qkv, w_out):
    KP, in_maps = _prep_inputs(hidden_states, attention_mask, w_qkv, w_out)
    if KP not in _NC_CACHE:
        _NC_CACHE[KP] = build_kernel(KP)
    nc = _NC_CACHE[KP]
    res = bass_utils.run_bass_kernel_spmd(nc, in_maps,
                                          core_ids=list(range(NCORES)))
    out = np.empty((B, S, H), dtype=np.float32)
    for b in range(B):
        acc = res.results[b * CPB]["outT"].astype(np.float32).copy()
        for c in range(b * CPB + 1, (b + 1) * CPB):
            acc += res.results[c]["outT"]
        out[b] = acc.T
    return out
